# revision 1
# baseline (speedup 1.0000x reference)
"""NonLocal2D (attention) block on 8 trn2 NeuronCores.

Sharding: core c -> batch n = c//2, query-half qh = c%2 (2048 of the 4096
spatial positions). Each core receives the full x[n] (so phi/g are computed
locally -- no collectives) plus its own query slice, and produces
out[n][:, qh*2048:(qh+1)*2048].

Per-core dataflow (layouts chosen so no transposes are ever needed):
  theta:    [CI=128, Q]  = wthT-chunks (lhsT) @ xt-chunks (rhs)     [PE]
  phi:      [CI=128, N]  = wphT-chunks (lhsT) @ xb-chunks (rhs)     [PE]
  g^T:      [s, CI] tiles = xb-chunks (lhsT) @ wgT-chunks (rhs)     [PE]
  scores^T: [s=128, q=1024] = phi-tile (lhsT) @ theta (rhs)         [PE -> PSUM f32]
  B = exp(SCALE*scores^T) -> bf16 SBUF (no max-sub: |scaled| < ~30) [ACT]
  y^T += gT-tile (lhsT) @ B   (PSUM accumulate over 32 s-tiles)     [PE]
  denom: F_j = sum of 8 B-tiles (bf16 adds, DVE/GPSIMD split), then
         d = ones (lhsT) @ F_j  (PSUM accumulate over j)            [PE]
  y_norm^T = y^T * (1/d) -> bf16                                    [DVE]
  r^T = woT-chunk (lhsT) @ y_norm^T ; out = x + r^T                 [PE, DVE]
"""

import numpy as np
import ml_dtypes

import concourse.bass as bass
import concourse.mybir as mybir
import concourse.tile as tile
from concourse import bacc
from concourse.bass import ts
from concourse.bass_utils import run_bass_kernel_spmd

BF16 = mybir.dt.bfloat16
F32 = mybir.dt.float32
AF = mybir.ActivationFunctionType
ALU = mybir.AluOpType

C = 256          # in channels
CI = 128         # inter channels
NB = 4           # batch
N = 4096         # H*W
Q = 2048         # queries per core
NCORES = 8
SCALE = float(128 ** 0.5)   # reference divides by d**-0.5

_CACHE: dict = {}


def _build(flags):
    bth_nz, bph_nz, bg_nz, bo_nz = flags
    nc = bacc.Bacc("TRN2", target_bir_lowering=False, debug=False)

    d = {}
    d["xb"] = nc.dram_tensor("xb", [2, 128, N], BF16, kind="ExternalInput").ap()
    d["xt"] = nc.dram_tensor("xt", [2, 128, Q], BF16, kind="ExternalInput").ap()
    d["xq"] = nc.dram_tensor("xq", [2, 128, Q], F32, kind="ExternalInput").ap()
    d["wthT"] = nc.dram_tensor("wthT", [2, 128, CI], BF16, kind="ExternalInput").ap()
    d["wphT"] = nc.dram_tensor("wphT", [2, 128, CI], BF16, kind="ExternalInput").ap()
    d["wgT"] = nc.dram_tensor("wgT", [2, 128, CI], BF16, kind="ExternalInput").ap()
    d["woT"] = nc.dram_tensor("woT", [128, C], BF16, kind="ExternalInput").ap()
    d["bth"] = nc.dram_tensor("bth", [128, 1], F32, kind="ExternalInput").ap() if bth_nz else None
    d["bph"] = nc.dram_tensor("bph", [128, 1], F32, kind="ExternalInput").ap() if bph_nz else None
    d["bg"] = nc.dram_tensor("bg", [1, CI], F32, kind="ExternalInput").ap() if bg_nz else None
    d["bo"] = nc.dram_tensor("bo", [2, 128, 1], F32, kind="ExternalInput").ap() if bo_nz else None
    d["out"] = nc.dram_tensor("out", [2, 128, Q], F32, kind="ExternalOutput").ap()

    with tile.TileContext(nc) as tc:
        _bass_body(tc, d)
    nc.compile()
    return nc


def _bass_body(tc, d):
    nc = tc.nc

    with (
        tc.tile_pool(name="const", bufs=1) as const,
        tc.tile_pool(name="acts", bufs=1) as acts,
        tc.tile_pool(name="outs", bufs=2) as outp,
    ):
        # ---- constants / weights ----
        ones_sb = const.tile([128, 128], BF16, tag="ones")
        nc.gpsimd.memset(ones_sb[:], 1.0)
        scratch = const.tile([128, 1], BF16, tag="scratch")
        # warm the exp table set early so the first real exp isn't +2.7us
        nc.scalar.activation(scratch[:], ones_sb[:, 0:1], AF.Exp, scale=1.0)
        wup_rhs = const.tile([128, 512], BF16, tag="wup_rhs")
        nc.gpsimd.memset(wup_rhs[:], 0.0)

        wth_sb = const.tile([128, 2, CI], BF16, tag="wth")
        wph_sb = const.tile([128, 2, CI], BF16, tag="wph")
        wg_sb = const.tile([128, 2, CI], BF16, tag="wg")
        wo_sb = const.tile([128, C], BF16, tag="wo")
        for kc in range(2):
            nc.sync.dma_start(out=wth_sb[:, kc, :], in_=d["wthT"][kc])
            nc.sync.dma_start(out=wph_sb[:, kc, :], in_=d["wphT"][kc])
            nc.sync.dma_start(out=wg_sb[:, kc, :], in_=d["wgT"][kc])
        nc.sync.dma_start(out=wo_sb[:], in_=d["woT"][:])
        bth_sb = bph_sb = bg_sb = bo_sb = None
        if d["bth"] is not None:
            bth_sb = const.tile([128, 1], F32, tag="bth")
            nc.sync.dma_start(out=bth_sb[:], in_=d["bth"][:])
        if d["bph"] is not None:
            bph_sb = const.tile([128, 1], F32, tag="bph")
            nc.sync.dma_start(out=bph_sb[:], in_=d["bph"][:])
        if d["bg"] is not None:
            bg_sb = const.tile([1, CI], F32, tag="bg")
            nc.sync.dma_start(out=bg_sb[:], in_=d["bg"][:])
        if d["bo"] is not None:
            bo_sb = const.tile([128, 2, 1], F32, tag="bo")
            for oc in range(2):
                nc.sync.dma_start(out=bo_sb[:, oc, :], in_=d["bo"][oc])

        th_sb = acts.tile([128, Q], BF16, tag="th")
        ph_sb = acts.tile([128, N], BF16, tag="ph")
        gT_sb = acts.tile([128, 32 * CI], BF16, tag="gT")  # tile i at cols [128i, 128i+128)

        # ---- phase pools with overlapping (non-nested) lifetimes ----
        xin_cm = tc.tile_pool(name="xin", bufs=1)
        xin = xin_cm.__enter__()
        # xt first (theta is the first projection); x spread across the
        # three DMA-capable queues so the fill isn't one-queue serial
        # chunk order follows first use: xt-ch0 (theta-hh0), xb-ch0 (phi-hh0),
        # xt-ch1 (theta-hh1), then the rest of xb; round-robin over the three
        # DMA-capable queues
        xt_sb = xin.tile([128, 2, Q], BF16, tag="xt")
        xb_sb = xin.tile([128, 2, N], BF16, tag="xb")
        qs = [nc.sync, nc.scalar, nc.gpsimd]
        transfers = (
            [("xt", kc, 0) for kc in range(2)]
            + [("xb", kc, 0) for kc in range(2)]
            + [("xt", kc, 1) for kc in range(2)]
            + [("xb", kc, ch) for ch in range(1, 4) for kc in range(2)]
        )
        for qi, (which, kc, ch) in enumerate(transfers):
            sb, dr = (xt_sb, d["xt"]) if which == "xt" else (xb_sb, d["xb"])
            qs[qi % 3].dma_start(
                out=sb[:, kc, ts(ch, 1024)], in_=dr[kc][:, ts(ch, 1024)])

        def cast_out(dst_ap, src_psum, bias_part, bias_row):
            # PSUM f32 -> SBUF bf16, optionally + bias
            if bias_part is not None:
                nc.vector.tensor_scalar_add(dst_ap, src_psum, bias_part[:])
            elif bias_row is not None:
                # bias varies along CI (free); repeat across partitions and
                # across the 4 g-tiles packed in this 512-wide slice
                bcast = bass.AP(
                    tensor=bias_row.tensor,
                    offset=bias_row.offset,
                    ap=[[0, 128], [0, 4], [1, CI]],
                )
                nc.vector.tensor_tensor(dst_ap, src_psum, bcast, ALU.add)
            else:
                nc.vector.tensor_copy(dst_ap, src_psum)

        # ---- theta + first quarter of phi (enough for 8 s-tiles) ----
        with (
            tc.tile_pool(name="pj", bufs=2, space="PSUM") as pj,
            tc.tile_pool(name="wup", bufs=1, space="PSUM") as wup,
        ):
            # PE warm-up during the DMA fill: ~3.5us of dummy matmuls flips
            # the HAM clock gate to 8/8 before the first real matmul issues
            wps = wup.tile([128, 512], F32, tag="wps")
            for _ in range(8):
                nc.tensor.matmul(
                    wps[:, 0:256], ones_sb[:], wup_rhs[:, 0:256],
                    start=True, stop=True)

            def theta_round(hh):
                tp = pj.tile([128, 1024], F32, tag="pj", name=f"tp{hh}")
                for qc in range(2):
                    for kc in range(2):
                        nc.tensor.matmul(
                            tp[:, ts(qc, 512)],
                            wth_sb[:, kc, :],
                            xt_sb[:, kc, ts(hh * 2 + qc, 512)],
                            start=(kc == 0),
                            stop=(kc == 1),
                        )
                cast_out(th_sb[:, ts(hh, 1024)], tp[:], bth_sb, None)

            # theta-hh0 -> phi-hh0 -> theta-hh1, so the DVE cast chain for
            # the first exp never waits on a later DMA chunk
            theta_round(0)
            pp = pj.tile([128, 1024], F32, tag="pj")
            for qc in range(2):
                for kc in range(2):
                    nc.tensor.matmul(
                        pp[:, ts(qc, 512)],
                        wph_sb[:, kc, :],
                        xb_sb[:, kc, ts(qc, 512)],
                        start=(kc == 0),
                        stop=(kc == 1),
                    )
            cast_out(ph_sb[:, 0:1024], pp[:], bph_sb, None)
            theta_round(1)

        # ---- attention, software-pipelined against the remaining
        # projections: exp for s-tile i+4 is emitted ahead of the y-matmuls
        # of tile i, and the first 4 score/exp pairs precede the phi tail and
        # the whole g^T phase (PE's in-order queue would otherwise park the
        # first exp behind them). phi-tail and g^T borrow the yps PSUM banks
        # (the y accumulation's start=True clears them afterwards).
        fF = [
            acts.tile([128, Q], BF16, tag=f"F{j}", name=f"F{j}")
            for j in range(4)
        ]
        ypsp_cm = tc.tile_pool(name="yps", bufs=1, space="PSUM")
        ypsp = ypsp_cm.__enter__()
        yps = ypsp.tile([128, Q], F32, tag="yps")
        scp_cm = tc.tile_pool(name="scp", bufs=2, space="PSUM")
        scp = scp_cm.__enter__()
        bp_cm = tc.tile_pool(name="bp", bufs=1)
        bp = bp_cm.__enter__()
        Bt = {}

        def sc_exp(i):
            # one B tile per s-tile (no slot reuse): the exp op then never
            # carries a WAR wait on the DVE fold, keeping the scalar queue
            # free of EVENT_SEMAPHORE instructions
            B = bp.tile([128, Q], BF16, tag=f"B{i}", name=f"B{i}")
            Bt[i] = B
            for h in range(2):
                sc = scp.tile([128, 1024], F32, tag="sc")
                for qc in range(2):
                    nc.tensor.matmul(
                        sc[:, ts(qc, 512)],
                        ph_sb[:, ts(i, 128)],
                        th_sb[:, ts(h * 2 + qc, 512)],
                        start=True,
                        stop=True,
                    )
                nc.scalar.activation(B[:, ts(h, 1024)], sc[:], AF.Exp, scale=SCALE)
            # stride-4 accumulator assignment: all four fold chains take
            # their last tile at i in {28..31}, so the fold tail past the
            # last exp is one add per chain
            j = i % 4
            if i < 4:
                nc.vector.tensor_copy(fF[j][:], B[:])
            else:
                nc.vector.tensor_add(fF[j][:], fF[j][:], B[:])

        for i in range(8):
            sc_exp(i)

        # phi tail (tiles 8..31) into borrowed yps banks
        for hh in range(1, 4):
            pp = yps[:, ts(hh % 2, 1024)]
            for qc in range(2):
                for kc in range(2):
                    nc.tensor.matmul(
                        pp[:, ts(qc, 512)],
                        wph_sb[:, kc, :],
                        xb_sb[:, kc, ts(hh * 2 + qc, 512)],
                        start=(kc == 0),
                        stop=(kc == 1),
                    )
            cast_out(ph_sb[:, ts(hh, 1024)], pp[:], bph_sb, None)

        # g^T projection, also into borrowed yps banks
        for b in range(8):
            gp = yps[:, 512 * (b % 4):512 * (b % 4) + 512]
            for sj in range(4):
                st = b * 4 + sj
                for kc in range(2):
                    nc.tensor.matmul(
                        gp[:, ts(sj, 128)],
                        xb_sb[:, kc, ts(st, 128)],
                        wg_sb[:, kc, :],
                        start=(kc == 0),
                        stop=(kc == 1),
                    )
            cast_out(gT_sb[:, ts(b, 512)], gp[:], None, bg_sb)

        # ---- main loop ----
        if True:
            for i in range(32):
                if i < 24:
                    sc_exp(i + 8)
                B = Bt[i]
                for h in range(2):
                    for qc in range(2):
                        nc.tensor.matmul(
                            yps[:, ts(h * 2 + qc, 512)],
                            gT_sb[:, ts(i, 128)],
                            B[:, ts(h * 2 + qc, 512)],
                            start=(i == 0),
                            stop=(i == 31),
                        )
            bp_cm.__exit__(None, None, None)
            scp_cm.__exit__(None, None, None)

            # ---- tail, pipelined per 512-wide q-chunk:
            # d (4 accumulating MMs) -> 1/d (approx) -> y*1/d -> out-proj ->
            # +x residual -> DMA out.  (exact `reciprocal` would be an
            # 8-cycle/elem iterative divide; approx_fast is ~18 bits, plenty)
            with (
                tc.tile_pool(name="dps", bufs=2, space="PSUM") as dpsp,
                tc.tile_pool(name="rps", bufs=2, space="PSUM") as rps,
            ):
                for qc in range(4):
                    xqts = []
                    for oc in range(2):
                        xqt = outp.tile(
                            [128, 512], F32, tag="xqt", name=f"xqt{qc}_{oc}")
                        nc.sync.dma_start(
                            out=xqt[:], in_=d["xq"][oc][:, ts(qc, 512)])
                        xqts.append(xqt)
                    dp = dpsp.tile([128, 512], F32, tag="dp")
                    for j in range(4):
                        nc.tensor.matmul(
                            dp[:],
                            ones_sb[:],
                            fF[j][:, ts(qc, 512)],
                            start=(j == 0),
                            stop=(j == 3),
                        )
                    rcp = outp.tile([128, 512], F32, tag="rcp")
                    nc.vector.reciprocal_approx_fast(rcp[:], dp[:])
                    ynt = outp.tile([128, 512], BF16, tag="ynt")
                    nc.vector.tensor_tensor(
                        ynt[:], yps[:, ts(qc, 512)], rcp[:], ALU.mult)
                    for oc in range(2):
                        rp = rps.tile([128, 512], F32, tag="rp")
                        nc.tensor.matmul(
                            rp[:],
                            wo_sb[:, ts(oc, 128)],
                            ynt[:],
                            start=True,
                            stop=True,
                        )
                        ot = outp.tile([128, 512], F32, tag="ot")
                        if bo_sb is not None:
                            nc.vector.scalar_tensor_tensor(
                                ot[:], rp[:], bo_sb[:, oc, :],
                                xqts[oc][:], ALU.add, ALU.add,
                            )
                        else:
                            nc.vector.tensor_tensor(
                                ot[:], rp[:], xqts[oc][:], ALU.add)
                        [nc.sync, nc.scalar][(2 * qc + oc) % 2].dma_start(
                            out=d["out"][oc][:, ts(qc, 512)], in_=ot[:])
            ypsp_cm.__exit__(None, None, None)
            xin_cm.__exit__(None, None, None)


def _prep_in_maps(inputs):
    bf = ml_dtypes.bfloat16
    x = np.ascontiguousarray(np.asarray(inputs["x"], dtype=np.float32))
    w_g = np.asarray(inputs["w_g"], np.float32)
    b_g = np.asarray(inputs["b_g"], np.float32)
    w_theta = np.asarray(inputs["w_theta"], np.float32)
    b_theta = np.asarray(inputs["b_theta"], np.float32)
    w_phi = np.asarray(inputs["w_phi"], np.float32)
    b_phi = np.asarray(inputs["b_phi"], np.float32)
    w_out = np.asarray(inputs["w_out"], np.float32)
    b_out = np.asarray(inputs["b_out"], np.float32)

    flags = (
        bool(np.any(b_theta)), bool(np.any(b_phi)),
        bool(np.any(b_g)), bool(np.any(b_out)),
    )
    wthT = np.ascontiguousarray(w_theta.T).astype(bf).reshape(2, 128, CI)
    wphT = np.ascontiguousarray(w_phi.T).astype(bf).reshape(2, 128, CI)
    wgT = np.ascontiguousarray(w_g.T).astype(bf).reshape(2, 128, CI)
    woT = np.ascontiguousarray(w_out.T).astype(bf)          # [CI, C]

    in_maps = []
    for c in range(NCORES):
        n, qh = c // 2, c % 2
        xr = x[n].reshape(C, N)
        xbc = xr.astype(bf)
        m = {
            "xb": np.ascontiguousarray(xbc.reshape(2, 128, N)),
            "xt": np.ascontiguousarray(
                xbc[:, qh * Q:(qh + 1) * Q].reshape(2, 128, Q)),
            "xq": np.ascontiguousarray(
                xr[:, qh * Q:(qh + 1) * Q].reshape(2, 128, Q)),
            "wthT": wthT, "wphT": wphT, "wgT": wgT, "woT": woT,
        }
        if flags[0]:
            m["bth"] = np.ascontiguousarray(b_theta.reshape(128, 1))
        if flags[1]:
            m["bph"] = np.ascontiguousarray(b_phi.reshape(128, 1))
        if flags[2]:
            m["bg"] = np.ascontiguousarray(b_g.reshape(1, CI))
        if flags[3]:
            m["bo"] = np.ascontiguousarray(b_out.reshape(2, 128, 1))
        in_maps.append(m)
    return flags, in_maps


def _get_nc(flags):
    if flags not in _CACHE:
        _CACHE[flags] = _build(flags)
    return _CACHE[flags]


def kernel(**inputs):
    flags, in_maps = _prep_in_maps(inputs)
    nc = _get_nc(flags)
    res = run_bass_kernel_spmd(nc, in_maps, list(range(NCORES)))
    out = np.empty((NB, C, N), np.float32)
    for c in range(NCORES):
        n, qh = c // 2, c % 2
        out[n][:, qh * Q:(qh + 1) * Q] = res.results[c]["out"].reshape(C, Q)
    return out.reshape(NB, C, 64, 64)


if __name__ == "__main__":
    x = np.random.randn(NB, C, 64, 64).astype(np.float32) * 0.1
    rng = np.random.default_rng(0)
    ins = {
        "x": x,
        "w_g": rng.normal(size=(CI, C)).astype(np.float32) * 0.01,
        "b_g": np.zeros(CI, np.float32),
        "w_theta": rng.normal(size=(CI, C)).astype(np.float32) * 0.01,
        "b_theta": np.zeros(CI, np.float32),
        "w_phi": rng.normal(size=(CI, C)).astype(np.float32) * 0.01,
        "b_phi": np.zeros(CI, np.float32),
        "w_out": rng.normal(size=(C, CI)).astype(np.float32) * 0.01,
        "b_out": np.zeros(C, np.float32),
    }
    o = kernel(**ins)
    print("ok", o.shape, o.dtype)

